# revision 51
# baseline (speedup 1.0000x reference)
"""Trainium2 Bass kernel for causal self-attention with PoPE.

Reference computation (B=2, T=2048, C=1024, H=16, D=64):
  qkv = x @ w_attn.T ; split q,k,v ; heads
  mu_q = softplus(q); mu_k = softplus(k)
  q_real = mu_q * cos(t w); q_imag = mu_q * sin(t w)
  k_real = mu_k * cos(t w + d); k_imag = mu_k * sin(t w + d)   [d = clip(delta)]
  att = softmax_causal((q_real k_real + q_imag k_imag)/sqrt(D))
  y = att @ v ; out = y @ w_proj.T

Sharding: 8 cores = 2 batches x 4 head-groups (4 heads each). Each core
computes its batch's QKV for its heads, attention, and a partial c_proj
(its heads' input-channel rows of w_proj). Host sums the 4 partials per
batch.

Per-core dataflow (all matmuls float32r: full PE rate at free dim >=
256, ~FP22 mantissa):
  xT   [c, t]     x[b]^T, c on partitions (8 tiles of 128)
  qk_h [128, t]   rows 0:64 = q_h, 64:128 = k_h (d-major), psum
  mu_h = ln(exp(qk_h)+1)   (ACT, exp in-place on psum; same table set
                            as the attention exp -> no table switches)
  Qt_h [128, t]   rows 0:64 mu_q*cos(tw)/8, rows 64:128 mu_q*sin(tw)/8
  Kt_h [128, t]   rows 0:64 mu_k*cos(tw+d), rows 64:128 mu_k*sin(tw+d)
  S^T  [tk, tq]   = Kt^T @ Qt (single K=128 matmul per 128x512 block,
                   two tq blocks share one 2-bank psum tile)
  P = exp(S^T)    no max subtraction (scores bounded ~6); causal mask as
                  0/1 multiply on the 16 diagonal blocks only (gpsimd)
  y_aug^T [96,tq] += V_aug[tk]^T @ P : V_aug = [V | ones | zeros] so row
                  64 of the psum accumulates the softmax denominator
  normalize       recip(denom) -> PE outer-product broadcast -> multiply
  c_proj          out[t,e] psum += y_t[c,t]^T @ w_projT[c,e]

Attention loops j-pairs outer so only 2 y-psum banks are live, leaving
room to double-buffer the 2-bank S tiles (fewer, wider ACT exp ops).
"""

import math
import os
import sys

import numpy as np

for _p in ("/opt/trn_rl_repo",):
    if _p not in sys.path and os.path.isdir(_p):
        sys.path.insert(0, _p)

import concourse.tile as tile
from concourse import bacc
from concourse import mybir
from concourse import bass_utils

B, T, C = 2, 2048, 1024
H, D = 16, 64
BASE = 10000.0
N_CORES = 8
HPC = 4  # heads per core
NCT = 8  # c tiles (1024/128)
NTT = 16  # t tiles of 128

F32 = mybir.dt.float32
F32R = mybir.dt.float32r
AF = mybir.ActivationFunctionType


def build_module():
    nc = bacc.Bacc(
        "TRN2", target_bir_lowering=False, debug=False, num_devices=N_CORES
    )

    xT_d = nc.dram_tensor("xT", (NCT, 128, T), F32R, kind="ExternalInput").ap()
    wqk_d = nc.dram_tensor("wqk", (NCT, 128, 512), F32R, kind="ExternalInput").ap()
    wv_d = nc.dram_tensor("wv", (NCT, 128, 256), F32R, kind="ExternalInput").ap()
    w2t_d = nc.dram_tensor("w2t", (2, 128, 1024), F32R, kind="ExternalInput").ap()
    trig_d = nc.dram_tensor("trig", (128, T), F32, kind="ExternalInput").ap()
    ab_d = nc.dram_tensor("ab", (HPC, 128, T), F32, kind="ExternalInput").ap()
    cmask_d = nc.dram_tensor("cmask", (128, 128), F32, kind="ExternalInput").ap()
    out_d = nc.dram_tensor("out", (NTT, 128, 1024), F32, kind="ExternalOutput").ap()

    idm = list(range(32))

    with tile.TileContext(nc) as tc:
        with (
            tc.tile_pool(name="persist", bufs=1) as persist,
            tc.tile_pool(name="mupool", bufs=1) as mupool,
            tc.tile_pool(name="ps2", bufs=2, space="PSUM") as ps2,
            tc.tile_pool(name="ps1", bufs=4, space="PSUM") as ps1,
        ):
            # persistent tiles (live across phases)
            v_aug = persist.tile([128, NTT, HPC, 65], F32R)
            ones_t = persist.tile([128, 128], F32R)
            nc.vector.memset(ones_t.bitcast(F32), 1.0)
            # per head slab: cols 0:64 = V, col 64 = ones (the y matmul
            # then accumulates the softmax denominator in psum row 64)
            nc.vector.memset(
                v_aug.rearrange("p a b c -> p (a b) c")[:, :, 64:65].bitcast(F32),
                1.0,
            )

            trig = persist.tile([128, T], F32)
            nc.gpsimd.dma_start(trig, trig_d)

            mu = [mupool.tile([128, T], F32, name=f"mu{h}") for h in range(HPC)]

            # ---------------- Phase A: QKV projection ----------------
            with tc.tile_pool(name="phA", bufs=1) as pha:
                xT = pha.tile([128, NCT, T], F32R)
                wv = pha.tile([128, NCT, 256], F32R)
                wqk_pool = tc.tile_pool(name="wqkp", bufs=1)
                wqkp = wqk_pool.__enter__()
                wqk = wqkp.tile([128, NCT, 512], F32R)
                nc.scalar.dma_start(wqk, wqk_d.rearrange("o p e -> p o e"))
                engs = [nc.sync, nc.gpsimd, nc.scalar]
                for o in range(NCT):
                    engs[o % 3].dma_start(xT[:, o, :], xT_d[o])
                nc.sync.dma_start(wv, wv_d.rearrange("o p e -> p o e"))

                # q,k per head: 2-bank psum [128, 1024] per tb-pair.
                # softplus = ln(exp(x)+1): exp lands in mu, ln runs in
                # place afterwards, batched so the ACT table isn't
                # reloaded between alternating Exp/Ln ops.
                exp_handles = {}
                for h in range(HPC):
                    for tbp in range(2):
                        ps = ps2.tile([128, 1024], F32, tag="s2", name="ps_qk")
                        base = tbp * 1024
                        for c in range(NCT):
                            for half in range(2):
                                o0 = half * 512
                                nc.tensor.matmul(
                                    ps[:, o0 : o0 + 512],
                                    lhsT=wqk[:, c, h * 128 : (h + 1) * 128],
                                    rhs=xT[:, c, base + o0 : base + o0 + 512],
                                    start=(c == 0),
                                    stop=(c == NCT - 1),
                                )
                        exp_handles[(h, tbp)] = nc.scalar.activation(
                            mu[h][:, base : base + 1024], ps, AF.Exp
                        )
                from concourse.tile_rust import add_dep_helper

                for h in range(HPC):
                    for tbp in range(2):
                        base = tbp * 1024
                        ln = nc.scalar.activation(
                            mu[h][:, base : base + 1024],
                            mu[h][:, base : base + 1024],
                            AF.Ln,
                            bias=1.0,
                        )
                        # order Lns after the 2-head group's last Exp so the
                        # ACT table isn't reloaded between every Exp/Ln pair
                        grp_last = exp_handles[(h, 1)]
                        add_dep_helper(
                            ln.ins,
                            grp_last.ins,
                            sync=False,
                            reason="group softplus lns after exps (ACT tables)",
                        )

                wqk_pool.__exit__(None, None, None)

                # V in [t, e] layout -> v_aug columns 0:64 per head
                for tt in range(NTT):
                    psv = ps1.tile([128, 512], F32, tag="mm", name="ps_v")
                    for c in range(NCT):
                        nc.tensor.matmul(
                            psv[:, 0:256],
                            lhsT=xT[:, c, tt * 128 : (tt + 1) * 128],
                            rhs=wv[:, c, :],
                            start=(c == 0),
                            stop=(c == NCT - 1),
                        )
                    nc.vector.tensor_copy(
                        out=v_aug[:, tt, :, 0:64],
                        in_=psv[:, 0:256].rearrange("p (h e) -> p h e", h=HPC),
                    )

            # ------------- Phase B: attention, j-pairs outer -------------
            ytp_ctx = tc.tile_pool(name="ytp", bufs=1)
            ytp = ytp_ctx.__enter__()
            y_t = ytp.tile([128, 2, T], F32R)
            w2 = ytp.tile([128, 2, 1024], F32R)
            nc.sync.dma_start(w2, w2t_d.rearrange("o p e -> p o e"))
            with (
                tc.tile_pool(name="phB", bufs=1) as phb,
                tc.tile_pool(name="abp", bufs=2) as abp,
                tc.tile_pool(name="qp", bufs=3) as qp,
                tc.tile_pool(name="kp", bufs=3) as kp,
                tc.tile_pool(name="swp", bufs=2) as swp,
                tc.tile_pool(name="pp", bufs=7) as pp,
                tc.tile_pool(name="smalls", bufs=3) as smalls,
            ):
                cmask = phb.tile([128, 128], F32)
                nc.gpsimd.dma_start(cmask, cmask_d)
                for h in range(HPC):
                    abh = abp.tile([128, T], F32, tag="ab", name="abh")
                    nc.sync.dma_start(abh, ab_d[h])
                    qt = qp.tile([128, T], F32R, tag="qt", name="qt")
                    kt = kp.tile([128, T], F32R, tag="kt", name="kt")
                    # cross-partition moves through f32 scratch (shuffle
                    # can't write f32r; TensorTensor needs aligned bases)
                    musw = swp.tile([128, T], F32, tag="musw", name="musw")
                    for hb in range(2):
                        ts_ = slice(hb * 1024, hb * 1024 + 1024)
                        nc.vector.stream_shuffle(
                            musw[64:128, ts_], mu[h][0:64, ts_], idm
                        )
                        nc.vector.stream_shuffle(
                            musw[0:64, ts_], mu[h][64:128, ts_], idm
                        )
                        # shuffle-dependent halves on DVE (fast, critical
                        # path); independent halves on gpsimd (2x slower but
                        # off the critical path)
                        nc.vector.tensor_mul(
                            kt[0:64, ts_], musw[0:64, ts_], abh[0:64, ts_]
                        )
                        nc.gpsimd.tensor_mul(
                            kt[64:128, ts_], mu[h][64:128, ts_], abh[64:128, ts_]
                        )
                        nc.gpsimd.tensor_mul(
                            qt[0:64, ts_], mu[h][0:64, ts_], trig[0:64, ts_]
                        )
                        nc.vector.tensor_mul(
                            qt[64:128, ts_], musw[64:128, ts_], trig[64:128, ts_]
                        )

                    for jp in range(2):
                        j0, j1 = 2 * jp, 2 * jp + 1
                        yps = [
                            ps1.tile([128, 512], F32, tag="mm", name=f"ps_y{jj}")
                            for jj in range(2)
                        ]
                        imax = 4 * j1 + 3
                        for i in range(imax + 1):
                            jlo = i // 4  # lowest valid j for this tk tile
                            r = i % 4
                            jset = [j for j in (j0, j1) if j >= jlo]
                            # for the diagonal block (j == jlo) only columns
                            # >= 128*r can be causally valid: narrow the S
                            # matmul, exp and y matmul to that range; the
                            # skipped psum columns get no contribution from
                            # this tk tile, which is exactly correct.
                            sps = ps2.tile([128, 1024], F32, tag="s2", name="ps_s")
                            for j in jset:
                                o0 = (j - j0) * 512
                                lo = 128 * r if j == jlo else 0
                                nc.tensor.matmul(
                                    sps[:, o0 + lo : o0 + 512],
                                    lhsT=kt[:, i * 128 : (i + 1) * 128],
                                    rhs=qt[:, j * 512 + lo : (j + 1) * 512],
                                    start=True,
                                    stop=True,
                                )
                            p_sb = pp.tile([128, 1024], F32R, tag="p", name="p_sb")
                            c0 = (jset[0] - j0) * 512 + (
                                128 * r if jset[0] == jlo else 0
                            )
                            c1 = (jset[-1] - j0) * 512 + 512
                            nc.scalar.activation(
                                p_sb[:, c0:c1], sps[:, c0:c1], AF.Exp
                            )
                            if jlo in (j0, j1):
                                # mask only the 128-wide diagonal strip
                                boff = (jlo - j0) * 512 + 128 * r
                                nc.gpsimd.tensor_mul(
                                    p_sb[:, boff : boff + 128],
                                    p_sb[:, boff : boff + 128],
                                    cmask,
                                )
                            for j in jset:
                                o0 = (j - j0) * 512
                                lo = 128 * r if j == jlo else 0
                                nc.tensor.matmul(
                                    yps[j - j0][0:65, lo:512],
                                    lhsT=v_aug[:, i, h, :],
                                    rhs=p_sb[:, o0 + lo : o0 + 512],
                                    start=(i == 0),
                                    stop=(i == 4 * j + 3),
                                )
                        for jj, j in ((0, j0), (1, j1)):
                            # reciprocal directly on psum row 64 (same start
                            # partition for in and out keeps the ISA happy)
                            rc = smalls.tile([128, 512], F32R, tag="rc", name="rc")
                            with nc.allow_low_precision(
                                reason="f32r (~fp22) reciprocal of softmax denom"
                            ):
                                nc.vector.reciprocal(
                                    rc[64:65, :], yps[jj][64:65, :]
                                )
                            # broadcast across partitions via PE outer
                            # product: ones[1,128].T @ rc[1,512] -> psum
                            bps = ps1.tile([128, 512], F32, tag="mm", name="ps_bc")
                            nc.tensor.matmul(
                                bps,
                                lhsT=ones_t[64:65, :],
                                rhs=rc[64:65, :],
                                start=True,
                                stop=True,
                            )
                            bc = smalls.tile([128, 512], F32, tag="bc", name="bc")
                            nc.vector.tensor_copy(out=bc, in_=bps)
                            if h % 2 == 0:
                                nc.vector.tensor_mul(
                                    y_t[0:64, h // 2, j * 512 : (j + 1) * 512],
                                    yps[jj][0:64, :],
                                    bc[0:64, :],
                                )
                            else:
                                # odd heads land on partitions 64:128 of y_t
                                ysh = smalls.tile(
                                    [128, 512], F32, tag="ysh", name="ysh"
                                )
                                nc.vector.stream_shuffle(
                                    ysh[64:128, :], yps[jj][0:64, :], idm
                                )
                                nc.vector.tensor_mul(
                                    y_t[64:128, h // 2, j * 512 : (j + 1) * 512],
                                    ysh[64:128, :],
                                    bc[64:128, :],
                                )

            # ---------------- Phase C: output projection ----------------
            with tc.tile_pool(name="ostage", bufs=8) as ostage:
                for tt in range(NTT):
                    po = ps2.tile([128, 1024], F32, tag="s2", name="ps_o")
                    for ct in range(2):
                        for eh in range(2):
                            nc.tensor.matmul(
                                po[:, eh * 512 : eh * 512 + 512],
                                lhsT=y_t[:, ct, tt * 128 : (tt + 1) * 128],
                                rhs=w2[:, ct, eh * 512 : (eh + 1) * 512],
                                start=(ct == 0),
                                stop=(ct == 1),
                            )
                    ost = ostage.tile([128, 1024], F32, tag="o", name="ost")
                    if tt % 2 == 0:
                        nc.scalar.copy(ost, po)
                    else:
                        nc.vector.tensor_copy(out=ost, in_=po)
                    eng = nc.sync if tt % 2 == 0 else nc.gpsimd
                    eng.dma_start(out_d[tt], ost)

            ytp_ctx.__exit__(None, None, None)

    nc.compile()
    return nc


def make_inputs(x, w_attn, w_proj, delta):
    """Host-side prep: per-core input dicts (core = b*4 + g)."""
    x = np.asarray(x, dtype=np.float32)
    w_attn = np.asarray(w_attn, dtype=np.float32)
    w_proj = np.asarray(w_proj, dtype=np.float32)
    delta = np.asarray(delta, dtype=np.float32)

    inv_freq = 1.0 / (BASE ** (np.arange(D, dtype=np.float32) / D))
    t = np.arange(T, dtype=np.float32)
    freqs = t[:, None] * inv_freq[None, :]  # (T, D)
    scale = 1.0 / math.sqrt(D)
    cosTs = (np.cos(freqs).T * scale).astype(np.float32)  # (D, T)
    sinTs = (np.sin(freqs).T * scale).astype(np.float32)
    trig = np.concatenate([cosTs, sinTs], axis=0)  # (128, T)

    d = np.clip(delta, -2.0 * math.pi, 0.0)

    qw = w_attn[:C].reshape(H, D, C)
    kw = w_attn[C : 2 * C].reshape(H, D, C)
    vw = w_attn[2 * C :].reshape(H, D, C)

    # causal mask for the 128-wide diagonal strip: valid iff c >= tk
    tk = np.arange(128)[:, None]
    cc = np.arange(128)[None, :]
    cmask = (cc >= tk).astype(np.float32)

    in_maps = []
    for core in range(N_CORES):
        b, g = divmod(core, HPC)
        heads = range(HPC * g, HPC * g + HPC)

        xT = np.ascontiguousarray(x[b].T).reshape(NCT, 128, T)

        qk = np.stack(
            [np.concatenate([qw[h], kw[h]], axis=0) for h in heads], axis=0
        )  # (4, 128, C)
        wqk = np.ascontiguousarray(qk.transpose(2, 0, 1).reshape(C, 512)).reshape(
            NCT, 128, 512
        )
        wv = np.ascontiguousarray(
            vw[HPC * g : HPC * g + HPC].reshape(256, C).T
        ).reshape(NCT, 128, 256)
        w2t = np.ascontiguousarray(
            w_proj[:, 256 * g : 256 * (g + 1)].T
        ).reshape(2, 128, 1024)

        ab = np.stack(
            [
                np.concatenate(
                    [
                        np.cos(freqs + d[h][None, :]).T,
                        np.sin(freqs + d[h][None, :]).T,
                    ],
                    axis=0,
                ).astype(np.float32)
                for h in heads
            ],
            axis=0,
        )  # (4, 128, T)

        in_maps.append(
            {
                "xT": xT,
                "wqk": wqk,
                "wv": wv,
                "w2t": w2t,
                "trig": trig,
                "ab": ab,
                "cmask": cmask,
            }
        )
    return in_maps


_NC_CACHE = []


def _get_nc():
    if not _NC_CACHE:
        _NC_CACHE.append(build_module())
    return _NC_CACHE[0]


def kernel(x, w_attn, w_proj, delta, _trace=False):
    in_maps = make_inputs(x, w_attn, w_proj, delta)
    nc = _get_nc()
    res = None
    outs = None
    last_err = None
    for attempt in range(3):
        try:
            res = bass_utils.run_bass_kernel_spmd(
                nc, in_maps, core_ids=list(range(N_CORES)), trace=_trace
            )
            # jax results are async: force materialization inside the
            # retry so a transient NRT_EXEC_UNIT_UNRECOVERABLE (seen on
            # the first execution of a freshly-loaded NEFF) is caught
            outs = [
                np.asarray(r["out"]).reshape(T, C) for r in res.results
            ]
            break
        except Exception as e:
            last_err = e
            if "unrecoverable" not in str(e).lower() or attempt == 2:
                raise
            import time as _time

            _time.sleep(2.0)
    assert outs is not None, last_err
    if _trace:
        kernel.last_results = res
    full = np.zeros((B, T, C), dtype=np.float32)
    for core in range(N_CORES):
        full[core // HPC] += outs[core]
    return full
